# revision 28
# baseline (speedup 1.0000x reference)
import os
import sys

for _p in ("/opt/trn_rl_repo",):
    if os.path.isdir(_p) and _p not in sys.path:
        sys.path.insert(0, _p)

import numpy as np
import ml_dtypes

import concourse.bass as bass
import concourse.tile as tile
from concourse import bacc
from concourse import mybir
from concourse import bass_utils
from concourse.alu_op_type import AluOpType

BF16 = ml_dtypes.bfloat16
AF = mybir.ActivationFunctionType

S = 1560
DIM = 1536
NH = 12
HD = 128
CACHE = 4680
NCORES = 8
RPC = S // NCORES
EPS = 1e-6
LOCAL_ATTN_SIZE = 3
SINK_SIZE = 1
MAX_ATTN = 32760 if LOCAL_ATTN_SIZE == -1 else LOCAL_ATTN_SIZE * S

NKC = (CACHE + 127) // 128
TAIL = CACHE - (NKC - 1) * 128
QB = 390

RCHUNKS = [(0, 128), (128, 195)]

_CACHED = {}
LAST_RUNS = []


def _build_launch1():
    nc = bacc.Bacc("TRN2", target_bir_lowering=False, debug=False,
                   num_devices=NCORES, num_swdge_queues=4)
    f32, bf = mybir.dt.float32, mybir.dt.bfloat16

    xt_d = nc.dram_tensor("xt", [128, 12, S], bf, kind="ExternalInput")
    xh_d = nc.dram_tensor("xh", [128, 12, 780], bf, kind="ExternalInput")
    wc_d = nc.dram_tensor("wc", [5, 12, 128, 128], bf, kind="ExternalInput")
    out_d = nc.dram_tensor("qkvt", [5, 128, S], bf, kind="ExternalOutput")

    RB = 390
    with tile.TileContext(nc) as tc:
        with (
            tc.tile_pool(name="consts", bufs=1) as consts,
            tc.tile_pool(name="ps", bufs=2, space="PSUM") as psp,
            tc.tile_pool(name="stg", bufs=4) as stgp,
        ):
            wc = consts.tile([128, 5, 12, 128], bf)
            for cc in range(5):
                eng = (nc.sync, nc.scalar, nc.gpsimd)[cc % 3]
                eng.dma_start(wc[:, cc, :, :], wc_d.ap()[cc].rearrange(
                    "k p n -> p k n"))
            xt = consts.tile([128, 12, S], bf)
            xh = consts.tile([128, 12, 780], bf)
            ei = 0
            for hb in range(2):
                for kc in range(12):
                    eng = (nc.sync, nc.scalar, nc.gpsimd)[ei % 3]
                    ei += 1
                    eng.dma_start(xt[:, kc, hb * 780:(hb + 1) * 780],
                                  xt_d.ap()[:, kc, hb * 780:(hb + 1) * 780])
                if hb == 0:
                    for kc in range(0, 12, 2):
                        eng = (nc.sync, nc.scalar, nc.gpsimd)[ei % 3]
                        ei += 1
                        eng.dma_start(xh[:, kc:kc + 2, :],
                                      xh_d.ap()[:, kc:kc + 2, :])

            wsrc = consts.tile([128, 390], bf, name="wsrc")
            nc.vector.memset(wsrc[:], 0.0)
            for wu in range(3):
                wp = psp.tile([128, 512], f32, tag=f"pr{wu}", name="lpw")
                nc.tensor.matmul(wp[:, 0:RB], wsrc[:, :128], wsrc[:],
                                 start=True, stop=True)

            pieces = [(0, 0), (1, 0), (2, 0), (3, 0), (4, 0), (4, 1),
                      (0, 1), (1, 1), (2, 1), (3, 1),
                      (0, 2), (1, 2), (2, 2), (3, 2),
                      (0, 3), (1, 3), (2, 3), (3, 3)]
            for idx, (cc, rb) in enumerate(pieces):
                pr = psp.tile([128, 512], f32, tag=f"pr{idx % 4}", name="pr")
                src = xt[:, :, rb * RB:(rb + 1) * RB] if cc < 4 else \
                    xh[:, :, rb * RB:(rb + 1) * RB]
                for kc in range(12):
                    nc.tensor.matmul(
                        pr[:, 0:RB],
                        wc[:, cc, kc, :],
                        src[:, kc, :],
                        start=(kc == 0), stop=(kc == 11))
                ot = stgp.tile([128, RB], bf, tag=f"ot{idx % 4}", name="ot")
                if idx % 2 == 0:
                    nc.scalar.copy(ot[:], pr[:, 0:RB])
                else:
                    nc.vector.tensor_copy(ot[:], pr[:, 0:RB])
                eng = (nc.sync, nc.scalar, nc.gpsimd)[idx % 3]
                eng.dma_start(out_d.ap()[cc][:, rb * RB:(rb + 1) * RB], ot[:])

    nc.finalize()
    return nc


PACKS = [(0, 4), (4, 3), (7, 4), (11, 3), (14, 4), (18, 3),
         (21, 4), (25, 3), (28, 4), (32, 3), (35, 2)]
UNITS = [(0, 0), (0, 1), (0, 2), (0, 3), (1, 0), (1, 1)]


def _build_launch2():
    nc = bacc.Bacc("TRN2", target_bir_lowering=False, debug=False,
                   num_devices=NCORES, num_swdge_queues=4)
    f32, bf, f16 = mybir.dt.float32, mybir.dt.bfloat16, mybir.dt.float16

    qt_d = nc.dram_tensor("qt", [2, 128, S], bf, kind="ExternalInput")
    kt_d = nc.dram_tensor("kt", [2, 128, NKC * 128], bf, kind="ExternalInput")
    vt_d = nc.dram_tensor("vt", [2, 128, NKC, 128], bf, kind="ExternalInput")
    wo_d = nc.dram_tensor("wo", [2, 128, DIM], bf, kind="ExternalInput")
    out_d = nc.dram_tensor("outp", [2, 780, DIM], f16, kind="ExternalOutput")

    with tile.TileContext(nc) as tc:
        with (
            tc.tile_pool(name="consts", bufs=1) as consts,
            tc.tile_pool(name="ps", bufs=1, space="PSUM") as psp,
            tc.tile_pool(name="pt", bufs=3) as ptp,
            tc.tile_pool(name="padd", bufs=4) as paddp,
            tc.tile_pool(name="small", bufs=2) as smallp,
            tc.tile_pool(name="outs", bufs=4) as outsp,
        ):
            qt0 = consts.tile([128, S], bf, name="qt0")
            qt1 = consts.tile([128, S], bf, name="qt1")
            wo0 = consts.tile([128, DIM], bf, name="wo0")
            wo1 = consts.tile([128, DIM], bf, name="wo1")
            kts = [consts.tile([128, NKC * 128], bf, name=f"kt{lh}")
                   for lh in range(2)]
            vts = [consts.tile([128, NKC, 128], bf, name=f"vt{lh}")
                   for lh in range(2)]
            nc.sync.dma_start(kts[0][:, :512], kt_d.ap()[0][:, :512])
            nc.scalar.dma_start(kts[0][:, 512:1536], kt_d.ap()[0][:, 512:1536])
            nc.sync.dma_start(qt0[:, :780], qt_d.ap()[0][:, :780])
            nc.sync.dma_start(kts[0][:, 1536:3136], kt_d.ap()[0][:, 1536:3136])
            nc.scalar.dma_start(kts[0][:, 3136:], kt_d.ap()[0][:, 3136:])
            nc.scalar.dma_start(qt0[:, 780:], qt_d.ap()[0][:, 780:])
            nc.scalar.dma_start(qt1[:, :780], qt_d.ap()[1][:, :780])
            nc.gpsimd.dma_start(vts[0][:, :12, :], vt_d.ap()[0][:, :12, :])
            nc.gpsimd.dma_start(vts[0][:, 12:, :], vt_d.ap()[0][:, 12:, :])
            nc.gpsimd.dma_start(kts[1][:, :2368], kt_d.ap()[1][:, :2368])
            nc.gpsimd.dma_start(kts[1][:, 2368:], kt_d.ap()[1][:, 2368:])
            nc.scalar.dma_start(vts[1][:, :12, :], vt_d.ap()[1][:, :12, :])
            nc.scalar.dma_start(vts[1][:, 12:, :], vt_d.ap()[1][:, 12:, :])
            nc.scalar.dma_start(qt1[:, 780:], qt_d.ap()[1][:, 780:])
            nc.sync.dma_start(wo0[:], wo_d.ap()[0])
            nc.sync.dma_start(wo1[:], wo_d.ap()[1])
            qts = [qt0, qt1]
            wos = [wo0, wo1]

            ones128b = consts.tile([128, 128], bf)
            nc.vector.memset(ones128b[:], 1.0)
            sab = consts.tile([128, 6, QB], bf)
            sacc_a = consts.tile([128, 6, QB], f32)
            sacc_b = consts.tile([128, 6, QB], f32)
            o3u = consts.tile([128, 6, QB], f32)
            o3 = consts.tile([128, 6, QB], bf)

            wsrc = consts.tile([128, 512], bf, name="wsrc")
            nc.vector.memset(wsrc[:], 0.0)
            wdst = consts.tile([128, 8], bf, name="wdst")
            for wu in range(3):
                wp = psp.tile([128, 512], f32, tag="opsum", name="lpw")
                nc.tensor.matmul(wp[:, 0:512], wsrc[:, :128], wsrc[:],
                                 start=True, stop=True)
                if wu == 0:
                    nc.scalar.activation(out=wdst[:], in_=wp[:, 0:8],
                                         func=AF.Exp)

            GP = [(u, lh, qb, pi, j0, m)
                  for u, (lh, qb) in enumerate(UNITS)
                  for pi, (j0, m) in enumerate(PACKS)]
            NG = len(GP)
            live = {}
            opsums = {}

            def emit_qk(g):
                u, lh, qb, pi, j0, m = GP[g]
                tag = "lpA" if pi % 2 == 0 else "lpB"
                width = 2048 if pi % 2 == 0 else 1536
                lp = psp.tile([128, width], f32, tag=tag, name="lp")
                pt = ptp.tile([128, width], bf, tag=tag + "p", name="pt")
                qsl = qts[lh][:, qb * QB:(qb + 1) * QB]
                for t in range(m):
                    j = j0 + t
                    nc.tensor.matmul(
                        lp[:, t * 512:t * 512 + QB],
                        kts[lh][:, j * 128:(j + 1) * 128],
                        qsl,
                        start=True, stop=True)
                live[g] = (lp, pt)

            emit_qk(0)
            emit_qk(1)
            for g in range(NG):
                u, lh, qb, pi, j0, m = GP[g]
                lp, pt = live.pop(g)
                lpv = lp.rearrange("p (b c) -> p b c", c=512)[:, 0:m, 0:QB]
                ptv = pt.rearrange("p (b c) -> p b c", c=512)[:, 0:m, 0:QB]
                nc.scalar.activation(out=ptv, in_=lpv, func=AF.Exp)
                if u not in opsums:
                    opsums[u] = psp.tile([128, 512], f32, tag="opsum",
                                         name="opsum")
                opsum = opsums[u]
                if g + 2 < NG:
                    emit_qk(g + 2)
                for t in range(m):
                    j = j0 + t
                    nc.tensor.matmul(
                        opsum[:, 0:QB],
                        vts[lh][:, j, :],
                        pt[:, t * 512:t * 512 + QB],
                        start=(j == 0), stop=(j == NKC - 1))
                sa = sacc_a[:, u, :]
                sb = sacc_b[:, u, :]
                if m == 4:
                    p1 = paddp.tile([128, QB], bf, tag="padd", name="p1")
                    p2 = paddp.tile([128, QB], bf, tag="padd", name="p2")
                    nc.vector.tensor_tensor(
                        p1[:], pt[:, 0:QB], pt[:, 512:512 + QB],
                        AluOpType.add)
                    nc.vector.tensor_tensor(
                        p2[:], pt[:, 1024:1024 + QB],
                        pt[:, 1536:1536 + QB], AluOpType.add)
                    if pi == 0:
                        nc.vector.tensor_copy(sa, p1[:])
                        nc.gpsimd.tensor_copy(sb, p2[:])
                    else:
                        nc.vector.tensor_tensor(sa, sa, p1[:],
                                                AluOpType.add)
                        nc.gpsimd.tensor_tensor(sb, sb, p2[:],
                                                AluOpType.add)
                elif m == 3:
                    p1 = paddp.tile([128, QB], bf, tag="padd", name="p1")
                    nc.vector.tensor_tensor(
                        p1[:], pt[:, 0:QB], pt[:, 512:512 + QB],
                        AluOpType.add)
                    nc.vector.tensor_tensor(sa, sa, p1[:], AluOpType.add)
                    nc.vector.tensor_tensor(sa, sa, pt[:, 1024:1024 + QB],
                                            AluOpType.add)
                else:
                    nc.vector.tensor_tensor(sa, sa, pt[:, 0:QB],
                                            AluOpType.add)
                    nc.vector.tensor_tensor(
                        sa[0:TAIL, :], sa[0:TAIL, :],
                        pt[0:TAIL, 512:512 + QB], AluOpType.add)
                if pi == len(PACKS) - 1:
                    nc.scalar.copy(o3u[:, u, :], opsum[:, 0:QB])
                    nc.vector.tensor_tensor(sab[:, u, :], sacc_a[:, u, :],
                                            sacc_b[:, u, :], AluOpType.add)
                    dbc = psp.tile([128, QB], f32, tag="lpB", name="dbc")
                    nc.tensor.matmul(dbc[:], ones128b[:], sab[:, u, :],
                                     start=True, stop=True)
                    dsb = smallp.tile([128, QB], f32, tag="dsb", name="dsb")
                    nc.scalar.copy(dsb[:], dbc[:])
                    rbc = smallp.tile([128, QB], f32, tag="rbc", name="rbc")
                    nc.vector.reciprocal_approx_fast(out=rbc[:], in_=dsb[:])
                    nc.vector.tensor_tensor(o3[:, u, :], o3u[:, u, :],
                                            rbc[:], AluOpType.mult)

            QCH = [(0, 128), (128, 128), (256, 128), (384, 128),
                   (512, 128), (640, 128), (768, 12)]
            o3f = o3.rearrange("p u q -> p (u q)")
            pidx = 0
            for blk in range(2):
                for (q0, qn) in QCH:
                    tag = "lpA" if (pidx % 2 == 0) else "lpB"
                    po = psp.tile([128, 1536], f32, tag=tag, name="po")
                    for cc in range(3):
                        wsl = slice(cc * 512, (cc + 1) * 512)
                        if blk == 0:
                            nc.tensor.matmul(
                                po[:qn, wsl], o3f[:, q0:q0 + qn],
                                wos[0][:, wsl], start=True, stop=False)
                            nc.tensor.matmul(
                                po[:qn, wsl],
                                o3f[:, 4 * QB + q0:4 * QB + q0 + qn],
                                wos[1][:, wsl], start=False, stop=True)
                        else:
                            nc.tensor.matmul(
                                po[:qn, wsl], o3f[:, 780 + q0:780 + q0 + qn],
                                wos[0][:, wsl], start=True, stop=True)
                    outf = outsp.tile([128, 1536], f16, tag="outf",
                                      name="outf")
                    nc.scalar.copy(outf[:qn, 0:1024], po[:qn, 0:1024])
                    nc.vector.tensor_copy(outf[:qn, 1024:], po[:qn, 1024:])
                    for di, eng in enumerate((nc.sync, nc.scalar, nc.gpsimd)):
                        dsl = slice(di * 512, (di + 1) * 512)
                        eng.dma_start(out_d.ap()[blk][q0:q0 + qn, dsl],
                                      outf[:qn, dsl])
                    pidx += 1

    nc.finalize()
    return nc


def _cache_plan(current_start, global_end_index, local_end_index, s, kv_size,
                frame_seqlen):
    current_end = current_start + s
    sink_tokens = SINK_SIZE * frame_seqlen

    kind = np.zeros(kv_size, dtype=np.int64)
    idx = np.arange(kv_size, dtype=np.int64)

    if (LOCAL_ATTN_SIZE != -1 and current_end > global_end_index
            and s + local_end_index > kv_size):
        num_evicted = s + local_end_index - kv_size
        num_rolled = local_end_index - num_evicted - sink_tokens
        src0 = sink_tokens + num_evicted
        kind[sink_tokens:sink_tokens + num_rolled] = \
            kind[src0:src0 + num_rolled]
        idx[sink_tokens:sink_tokens + num_rolled] = \
            idx[src0:src0 + num_rolled]
        new_local_end = (local_end_index + current_end - global_end_index
                         - num_evicted)
    else:
        new_local_end = local_end_index + current_end - global_end_index
    local_start = new_local_end - s
    is_recompute = (current_end <= global_end_index) and (current_start > 0)
    write_start = max(local_start, sink_tokens) if is_recompute \
        else local_start
    off = max(0, write_start - local_start)
    wl = max(0, new_local_end - write_start)
    if wl > 0:
        kind[write_start:new_local_end] = 1
        idx[write_start:new_local_end] = off + np.arange(wl)

    if sink_tokens > 0:
        budget = MAX_ATTN - sink_tokens
        if budget > 0:
            lo = max(sink_tokens, new_local_end - budget)
            sel = np.concatenate([np.arange(sink_tokens),
                                  np.arange(lo, new_local_end)])
        else:
            sel = np.arange(sink_tokens)
    else:
        ws = max(0, new_local_end - MAX_ATTN)
        sel = np.arange(ws, new_local_end)

    k_kind, k_idx = kind[sel], idx[sel]
    old_rows = k_idx[k_kind == 0]
    new_rows = k_idx[k_kind == 1]
    return old_rows, new_rows


def _rope_tables(freqs_real, freqs_imag, f, h, w, start_frame, gq, gk):
    c = HD // 2
    c0 = c - 2 * (c // 3)
    c1 = c // 3
    fr = np.asarray(freqs_real, np.float32)
    fi = np.asarray(freqs_imag, np.float32)
    s = f * h * w
    assert s == S
    fidx = np.arange(s) // (h * w)
    hidx = (np.arange(s) // w) % h
    widx = np.arange(s) % w
    fr_pos = np.concatenate([
        fr[start_frame + fidx][:, :c0],
        fr[hidx][:, c0:c0 + c1],
        fr[widx][:, c0 + c1:c0 + 2 * c1],
    ], axis=1)
    fi_pos = np.concatenate([
        fi[start_frame + fidx][:, :c0],
        fi[hidx][:, c0:c0 + c1],
        fi[widx][:, c0 + c1:c0 + 2 * c1],
    ], axis=1)
    C1 = np.repeat(fr_pos, 2, axis=1)
    Sg = np.empty((s, HD), np.float32)
    Sg[:, 0::2] = -fi_pos
    Sg[:, 1::2] = fi_pos
    C = np.tile(C1, (1, NH))
    Sx = np.tile(Sg, (1, NH))
    gq = np.asarray(gq, np.float32)
    gk = np.asarray(gk, np.float32)
    gq_sw = gq.reshape(-1, 2)[:, ::-1].reshape(-1)
    gk_sw = gk.reshape(-1, 2)[:, ::-1].reshape(-1)
    return (C * gq[None, :], Sx * gq_sw[None, :],
            C * gk[None, :], Sx * gk_sw[None, :])


def kernel(x, cache_k, cache_v, freqs_real, freqs_imag,
           wq, bq, wk, bk, wv, bv, wo, bo, gq, gk,
           f_frames, height, width, current_start, global_end_index,
           local_end_index):
    global LAST_RUNS
    LAST_RUNS = []

    x = np.asarray(x, np.float32)
    cache_k = np.asarray(cache_k, np.float32)
    cache_v = np.asarray(cache_v, np.float32)
    wq = np.asarray(wq, np.float32)
    wk = np.asarray(wk, np.float32)
    wv = np.asarray(wv, np.float32)
    wo = np.asarray(wo, np.float32)
    bo = np.asarray(bo, np.float32)
    f = int(f_frames)
    h = int(height)
    w = int(width)
    current_start = int(current_start)
    global_end_index = int(global_end_index)
    local_end_index = int(local_end_index)

    assert x.shape == (1, S, DIM)
    for b in (bq, bk, bv):
        assert not np.any(np.asarray(b)), "nonzero qkv bias unsupported"

    frame_seqlen = h * w
    start_frame = current_start // frame_seqlen

    W_all = np.concatenate([wq, wk, wv], axis=1)
    xT = x[0].T.astype(BF16)
    xt_full = np.ascontiguousarray(
        xT.reshape(12, 128, S).transpose(1, 0, 2))

    nc1 = _CACHED.get("l1")
    if nc1 is None:
        nc1 = _CACHED["l1"] = _build_launch1()

    in_maps1 = []
    for c in range(NCORES):
        chunks = [4 * c + i for i in range(4)] + [32 + c // 2]
        wc = np.stack([
            np.ascontiguousarray(
                W_all[:, ch * 128:(ch + 1) * 128].reshape(12, 128, 128))
            for ch in chunks]).astype(BF16)
        half = c % 2
        in_maps1.append({
            "xt": xt_full,
            "xh": np.ascontiguousarray(
                xt_full[:, :, half * 780:(half + 1) * 780]),
            "wc": wc,
        })
    res1 = bass_utils.run_bass_kernel_spmd(nc1, in_maps1,
                                           core_ids=list(range(NCORES)))
    LAST_RUNS.append(res1)

    QKVT = np.empty((4608, S), BF16)
    for c in range(NCORES):
        o = res1.results[c]["qkvt"]
        for i in range(4):
            ch = 4 * c + i
            QKVT[ch * 128:(ch + 1) * 128] = o[i]
        sh = 32 + c // 2
        half = c % 2
        QKVT[sh * 128:(sh + 1) * 128, half * 780:(half + 1) * 780] = \
            o[4][:, 0:780]

    QT_raw = QKVT[0:DIM].astype(np.float32)
    KT_raw = QKVT[DIM:2 * DIM].astype(np.float32)
    VT_raw = QKVT[2 * DIM:]

    scale_q = (1.0 / np.sqrt((QT_raw * QT_raw).mean(0) + EPS)
               / np.sqrt(HD)).astype(np.float32)
    scale_k = (1.0 / np.sqrt((KT_raw * KT_raw).mean(0) + EPS))

    Cq, Sq, Ck, Sk = _rope_tables(freqs_real, freqs_imag, f, h, w,
                                  start_frame, gq, gk)
    QT_sw = QT_raw.reshape(DIM // 2, 2, S)[:, ::-1, :].reshape(DIM, S)
    KT_sw = KT_raw.reshape(DIM // 2, 2, S)[:, ::-1, :].reshape(DIM, S)
    Qt = ((QT_raw * Cq.T + QT_sw * Sq.T) * scale_q[None, :]).astype(BF16)
    Kt = ((KT_raw * Ck.T + KT_sw * Sk.T) * scale_k[None, :]).astype(BF16)
    QT = np.ascontiguousarray(Qt.reshape(NH, HD, S))
    KnewT = Kt.reshape(NH, HD, S)
    Vnew = np.ascontiguousarray(VT_raw.T)

    old_rows, new_rows = _cache_plan(current_start, global_end_index,
                                     local_end_index, S, cache_k.shape[1],
                                     frame_seqlen)
    n_keys = len(old_rows) + len(new_rows)
    assert n_keys == CACHE, f"unexpected key count {n_keys}"

    kth = np.zeros((NH, HD, NKC * 128), BF16)
    kth[:, :, 0:len(old_rows)] = \
        cache_k[0, old_rows].astype(BF16).transpose(1, 2, 0)
    kth[:, :, len(old_rows):CACHE] = KnewT[:, :, new_rows]
    V_pad = np.zeros((NKC * 128, DIM), BF16)
    V_pad[0:len(old_rows)] = \
        cache_v[0, old_rows].reshape(len(old_rows), DIM).astype(BF16)
    V_pad[len(old_rows):CACHE] = Vnew[new_rows]
    vth = np.ascontiguousarray(
        V_pad.reshape(NKC, 128, NH, HD).transpose(2, 1, 0, 3))
    woh = np.ascontiguousarray(wo.reshape(NH, HD, DIM)).astype(BF16)

    nc2 = _CACHED.get("l2")
    if nc2 is None:
        nc2 = _CACHED["l2"] = _build_launch2()

    in_maps2 = []
    for c in range(NCORES):
        g_lo = (3 * c) // 2
        g_hi = g_lo + 1
        if c % 2 == 0:
            g_full, g_half = g_lo, g_hi
        else:
            g_full, g_half = g_hi, g_lo
        qt_c = np.empty((2, HD, S), BF16)
        for i, g in enumerate((g_full, g_half)):
            if c % 2 == 0:
                qt_c[i] = QT[g]
            else:
                qt_c[i, :, 0:780] = QT[g][:, 780:1560]
                qt_c[i, :, 780:1560] = QT[g][:, 0:780]
        in_maps2.append({
            "qt": qt_c,
            "kt": np.ascontiguousarray(kth[[g_full, g_half]]),
            "vt": np.ascontiguousarray(vth[[g_full, g_half]]),
            "wo": np.ascontiguousarray(woh[[g_full, g_half]]),
        })
    res2 = bass_utils.run_bass_kernel_spmd(nc2, in_maps2,
                                           core_ids=list(range(NCORES)))
    LAST_RUNS.append(res2)

    out = np.zeros((S, DIM), np.float32)
    for c in range(NCORES):
        blk = res2.results[c]["outp"].astype(np.float32)
        if c % 2 == 0:
            out[0:780] += blk[0]
            out[780:1560] += blk[1]
        else:
            out[780:1560] += blk[0]
            out[0:780] += blk[1]
    out += bo.reshape(1, DIM)
    return out.reshape(1, S, DIM)

